# revision 37
# baseline (speedup 1.0000x reference)
"""Distributed Trainium2 (Bass/Tile) kernel for an enhanced ST-GCN layer.

Math (matches the jax reference):
  h   = LN(conv1d_t(x, k=3, pad=1) + tconv_b) * ln_g + ln_b ; relu
  res = x @ res_w.T + res_b
  xt  = (h + res) @ gcn_w.T          (res folded: xt += x @ (gcn_w@res_w).T + gcn_w@res_b)
  ew  = softmax(edge_weight); deg[v] = 1 + sum_{e: col=v} ew_e ; dinv = rsqrt(deg)
  out[v] = relu(dinv[v] * (sum_{e: col=v} ew_e * dinv[row_e] * xt[row_e]  + dinv[v]*xt[v]) + gcn_b)

Sharding: nodes split contiguously across 8 cores (2500/core, padded to 2560).
Each core computes its nodes' dense trunk in a channels-on-partitions layout
(fp32r matmuls), scales by dinv, transposes to node-major, AllGathers an fp8
copy of xs = dinv*xt (edge weights pre-scaled 2^13 to survive e4m3), then
gathers source rows for its (dest-sorted, bucketed) edges with gpsimd
dma_gather and reduces them with one-hot-style S matmuls; the fp32 self-term
and a 2^-13 descale are applied in the epilogue.
"""

import os
import numpy as np
import ml_dtypes

import concourse.bass as bass
import concourse.tile as tile
from contextlib import ExitStack
from concourse import bacc, mybir
from concourse.bass_utils import run_bass_kernel_spmd

F32 = mybir.dt.float32
F32R = mybir.dt.float32r
BF16 = mybir.dt.bfloat16
GF8 = mybir.dt.float8e4
I16 = mybir.dt.int16

NCORES = 8
N, E, T, CIN, CO = 20000, 160000, 8, 64, 128
PC = N // NCORES            # 2500 nodes per core
NP = 2560                   # padded per-core nodes (20 buckets x 128)
NBUK = NP // 128            # 20
NCH = NP // 512             # 5 node chunks of 512
TC = T * CO                 # 1024 features per node row

_PROG_CACHE = {}


# ---------------------------------------------------------------- host prep

def _prep(x, edge_index, edge_weight, tconv_w, tconv_b, ln_g, ln_b,
          res_w, res_b, gcn_w, gcn_b, cpb):
    cap = cpb * 128           # edge slots per bucket
    cc_per_core = NBUK * cpb  # chunks per core
    tot_ch = NCORES * cc_per_core

    row = np.asarray(edge_index[0], np.int64)
    col = np.asarray(edge_index[1], np.int64)
    ew = np.asarray(edge_weight, np.float32)

    core_of = col // PC
    rel = col - core_of * PC
    buk = rel // 128

    # slot arrays, global chunk order: core-major, bucket-major
    g_idx = np.zeros(tot_ch * 128, np.int16)           # gather row ids (pad 0)
    g_colr = np.full(tot_ch * 128, -1.0, np.float32)   # col rel to bucket (pad -1)
    g_ew = np.full(tot_ch * 128, -1e9, np.float32)     # raw edge weight (pad -1e9)

    order = np.lexsort((buk, core_of))
    srow, scol_rel, sew = row[order], (rel - buk * 128)[order], ew[order]
    scor, sbuk = core_of[order], buk[order]
    # per (core,bucket) group boundaries
    gid = scor * NBUK + sbuk
    counts = np.bincount(gid, minlength=NCORES * NBUK)
    assert counts.max() <= cap, f"bucket overflow: {counts.max()} > {cap}"
    starts = np.concatenate([[0], np.cumsum(counts)])[:-1]
    within = np.arange(len(order)) - starts[gid]
    slot = gid * cap + within
    # chunk-major AG layout: table row = chunk*(8*512) + core*512 + local%512
    sloc = srow % PC
    g_idx[slot] = ((sloc // 512) * (NCORES * 512) + (srow // PC) * 512
                   + sloc % 512).astype(np.int16)
    g_colr[slot] = scol_rel
    g_ew[slot] = sew

    # per-core device arrays
    in_maps = []
    w0t = np.ascontiguousarray(tconv_w[:, :, 0].T)  # [64,128]
    w1t = np.ascontiguousarray(tconv_w[:, :, 1].T)
    w2t = np.ascontiguousarray(tconv_w[:, :, 2].T)
    wstack01 = np.concatenate([w0t, w1t], axis=0)   # [128,128]
    w2thi = np.zeros((128, CO), np.float32); w2thi[64:] = w2t
    gcnwT = np.ascontiguousarray(gcn_w.T)           # [128,128]
    rw2 = gcn_w @ res_w                              # [128,64]
    rw2thi = np.zeros((128, CO), np.float32); rw2thi[64:] = rw2.T
    rb2 = (gcn_w @ res_b).astype(np.float32)        # [128]
    oneh = np.zeros((128, T, T), np.float32)
    for t in range(T):
        oneh[:, t, t] = 1.0
    # selrow[k, t*128+p] = 1 iff k == t  (K=8 broadcast matmul lhsT)
    selrow = np.zeros((T, T * 128), np.float32)
    for t in range(T):
        selrow[t, t * 128:(t + 1) * 128] = 1.0
    iota = np.tile(np.arange(128, dtype=np.float32), (128, 1))
    gcnb_row = np.tile(gcn_b.astype(np.float32), T)[None, :]   # [1,1024]

    ew_chunks = g_ew.reshape(tot_ch, 128).T          # [128, tot_ch]
    colr_chunks = g_colr.reshape(tot_ch, 128).T      # [128, tot_ch]
    # wrapped int16 idx per bucket: [16, cap/16] tiled to 128 partitions
    idx_b = g_idx.reshape(NCORES * NBUK, cap)
    idx_wrapped = np.stack([np.tile(b.reshape(-1, 16).T, (8, 1)) for b in idx_b])
    # [NCORES*NBUK, 128, cap/16] -> per core concat along free
    idx_wrapped = idx_wrapped.reshape(NCORES, NBUK, 128, cap // 16)
    idx_wrapped = np.ascontiguousarray(np.transpose(idx_wrapped, (0, 2, 1, 3))
                                       ).reshape(NCORES, 128, NBUK * cap // 16)

    x = np.asarray(x, np.float32)
    for c in range(NCORES):
        xs = np.zeros((T, CIN, NP), np.float32)
        xs[:, :, :PC] = x[:, c * PC:(c + 1) * PC, :].transpose(0, 2, 1)
        my = slice(c * cc_per_core, (c + 1) * cc_per_core)
        ew_mine = ew_chunks[:, my]
        ew_rest = np.concatenate([ew_chunks[:, :c * cc_per_core],
                                  ew_chunks[:, (c + 1) * cc_per_core:]], axis=1)
        in_maps.append(dict(
            x_in=xs,
            wstack01=wstack01, w2thi=w2thi, gcnwT=gcnwT, rw2thi=rw2thi,
            rb2col=rb2[None, :], oneh=oneh.reshape(128, T * T), selrow=selrow,
            iota=iota, gcnb_row=gcnb_row,
            lng=ln_g.astype(np.float32)[:, None], lnb=ln_b.astype(np.float32)[:, None],
            tcb=tconv_b.astype(np.float32)[:, None],
            ewl=np.ascontiguousarray(np.concatenate([ew_mine, ew_rest], axis=1)),
            colr=np.ascontiguousarray(colr_chunks[:, my]),
            gidx=np.ascontiguousarray(idx_wrapped[c]),
        ))
    flags = dict(
        gcnb_zero=bool(np.all(gcn_b == 0)),
        rb2_zero=bool(np.all(res_b == 0)),
    )
    return in_maps, flags


# ---------------------------------------------------------------- device code

def _build(cpb, gcnb_zero, rb2_zero):
    phase = int(os.environ.get("KPHASE", "4"))
    skips = set(os.environ.get("KSKIP", "").split(","))
    cap = cpb * 128
    cc_per_core = NBUK * cpb
    tot_ch = NCORES * cc_per_core

    nc = bacc.Bacc("TRN2", target_bir_lowering=False, debug=False,
                   num_devices=NCORES, num_swdge_queues=2)

    x_in = nc.dram_tensor("x_in", [T, CIN, NP], F32R, kind="ExternalInput")
    wstack01 = nc.dram_tensor("wstack01", [128, CO], F32R, kind="ExternalInput")
    w2thi = nc.dram_tensor("w2thi", [128, CO], F32R, kind="ExternalInput")
    gcnwT = nc.dram_tensor("gcnwT", [CO, CO], F32R, kind="ExternalInput")
    rw2thi = nc.dram_tensor("rw2thi", [128, CO], F32R, kind="ExternalInput")
    rb2col = nc.dram_tensor("rb2col", [1, CO], F32R, kind="ExternalInput")
    oneh = nc.dram_tensor("oneh", [128, T * T], F32R, kind="ExternalInput")
    selrow_d = nc.dram_tensor("selrow", [T, T * 128], F32R, kind="ExternalInput")
    iota_d = nc.dram_tensor("iota", [128, 128], F32, kind="ExternalInput")
    gcnb_row = nc.dram_tensor("gcnb_row", [1, TC], F32R, kind="ExternalInput")
    lng_d = nc.dram_tensor("lng", [CO, 1], F32, kind="ExternalInput")
    lnb_d = nc.dram_tensor("lnb", [CO, 1], F32, kind="ExternalInput")
    tcb_d = nc.dram_tensor("tcb", [CO, 1], F32, kind="ExternalInput")
    ewl_d = nc.dram_tensor("ewl", [128, tot_ch], F32, kind="ExternalInput")
    colr_d = nc.dram_tensor("colr", [128, cc_per_core], F32, kind="ExternalInput")
    gidx_d = nc.dram_tensor("gidx", [128, NBUK * cap // 16], I16, kind="ExternalInput")
    out_d = nc.dram_tensor("out", [NP, TC], F32, kind="ExternalOutput")

    with tile.TileContext(nc, num_cores=NCORES) as tc, ExitStack() as ctx:
        ctx.enter_context(nc.allow_low_precision(
            reason="fp32r rounding of matmul operands is intentional"))
        const = ctx.enter_context(tc.tile_pool(name="const", bufs=1))
        dram = ctx.enter_context(tc.tile_pool(name="dram", bufs=1, space="DRAM"))
        resi = ctx.enter_context(tc.tile_pool(name="resi", bufs=1))

        # ---- constants
        ws01 = const.tile([128, CO], F32R); nc.sync.dma_start(ws01[:], wstack01[:])
        w2hi = const.tile([128, CO], F32R); nc.sync.dma_start(w2hi[:], w2thi[:])
        gwt = const.tile([CO, CO], F32R); nc.sync.dma_start(gwt[:], gcnwT[:])
        rwhi = const.tile([128, CO], F32R); nc.sync.dma_start(rwhi[:], rw2thi[:])
        rb2t = const.tile([1, CO], F32R); nc.sync.dma_start(rb2t[:], rb2col[:])
        onet = const.tile([128, T, T], F32R); nc.sync.dma_start(onet[:], oneh[:].rearrange("p (a b) -> p a b", a=T))
        selr = const.tile([T, T * 128], F32R); nc.sync.dma_start(selr[:], selrow_d[:])
        iota_t = const.tile([128, 128], F32); nc.sync.dma_start(iota_t[:], iota_d[:])
        gbrow = const.tile([1, TC], F32R); nc.sync.dma_start(gbrow[:], gcnb_row[:])
        lng_t = const.tile([CO, 1], F32); nc.sync.dma_start(lng_t[:], lng_d[:])
        lnb_t = const.tile([CO, 1], F32); nc.sync.dma_start(lnb_t[:], lnb_d[:])
        tcb_t = const.tile([CO, 1], F32); nc.sync.dma_start(tcb_t[:], tcb_d[:])
        colr_t = const.tile([128, cc_per_core], F32); nc.sync.dma_start(colr_t[:], colr_d[:])
        gidx_t = const.tile([128, NBUK * cap // 16], I16); nc.sync.dma_start(gidx_t[:], gidx_d[:])
        ones_bf = const.tile([128, 1], BF16); nc.vector.memset(ones_bf[:], 1.0)
        ones_f1 = const.tile([1, 128], F32); nc.vector.memset(ones_f1[:], 1.0)
        ones512 = const.tile([1, 512], F32); nc.vector.memset(ones512[:], 1.0)
        eps_t = const.tile([T, 1], F32); nc.vector.memset(eps_t[:], 1e-5)
        one_t = const.tile([1, 1], F32); nc.vector.memset(one_t[:], 1.0)
        ew_soft = const.tile([128, cc_per_core], F32)
        dinv_pp = const.tile([128, NBUK], F32)
        # gcn_b broadcast [128, TC] (built below via K=1 matmuls) unless zero
        gcnbb = None if gcnb_zero else const.tile([128, TC], F32)

        # resident across phases
        s1T = [resi.tile([128, TC], F32, tag=f"s1T{b}", name=f"s1T{b}") for b in range(NBUK)]

        # ---- phase A: softmax + degree + dinv
        with tc.tile_pool(name="pha", bufs=2) as pha, \
             tc.tile_pool(name="pha1", bufs=1) as pha1, \
             tc.tile_pool(name="phaps", bufs=2, space="PSUM") as phaps:
            ewl_t = pha1.tile([128, tot_ch], F32)
            nc.sync.dma_start(ewl_t[:], ewl_d[:])
            ewx = pha1.tile([128, tot_ch], F32)
            nc.scalar.activation(ewx[:], ewl_t[:], mybir.ActivationFunctionType.Exp)
            rsum = pha1.tile([128, 1], F32)
            nc.vector.tensor_reduce(rsum[:], ewx[:], axis=mybir.AxisListType.X,
                                    op=mybir.AluOpType.add)
            ones_f = pha1.tile([128, 1], F32); nc.vector.memset(ones_f[:], 1.0)
            z_ps = phaps.tile([1, 1], F32, space="PSUM")
            nc.tensor.matmul(z_ps[:], lhsT=rsum[:], rhs=ones_f[:], start=True, stop=True)
            zinv = pha1.tile([1, 1], F32)
            nc.vector.reciprocal(zinv[:], z_ps[:])
            zb_ps = phaps.tile([128, 1], F32, space="PSUM")
            nc.tensor.matmul(zb_ps[:], lhsT=ones_f1[:], rhs=zinv[:],
                             start=True, stop=True)
            zinv_pp = pha1.tile([128, 1], F32)
            nc.vector.tensor_copy(zinv_pp[:], zb_ps[:])
            nc.vector.tensor_scalar(out=ew_soft[:], in0=ewx[:, :cc_per_core],
                                    scalar1=zinv_pp[:, :1], scalar2=8192.0,
                                    op0=mybir.AluOpType.mult,
                                    op1=mybir.AluOpType.mult)

            ones_t1 = pha1.tile([1, 1], F32)
            nc.vector.memset(ones_t1[:], 1.0)
            degcol = pha1.tile([128, NBUK], F32)
            for b in range(NBUK):
                dps = phaps.tile([1, 128], F32, space="PSUM", tag="dps")
                for j in range(cpb):
                    ch = b * cpb + j
                    s_t = pha.tile([128, 128], BF16, tag="sdeg")
                    nc.vector.tensor_scalar(
                        out=s_t[:], in0=iota_t[:],
                        scalar1=colr_t[:, ch:ch + 1], scalar2=ew_soft[:, ch:ch + 1],
                        op0=mybir.AluOpType.is_equal, op1=mybir.AluOpType.mult)
                    nc.tensor.matmul(dps[:], lhsT=ones_bf[:], rhs=s_t[:],
                                     start=(j == 0), stop=(j == cpb - 1))
                degsb = pha.tile([1, 128], F32, tag="degsb")
                nc.vector.tensor_copy(degsb[:], dps[:])
                dtp = phaps.tile([128, 1], F32, space="PSUM", tag="dtp")
                nc.tensor.transpose(dtp[:], degsb[:], ones_t1[:])
                nc.vector.tensor_copy(degcol[:, b:b + 1], dtp[:])
            # dinv = 1/sqrt(deg + 1)
            dsq = pha1.tile([128, NBUK], F32)
            nc.scalar.activation(dsq[:], degcol[:], mybir.ActivationFunctionType.Sqrt,
                                 bias=ones_f[:, :1], scale=1.0 / 8192.0)
            nc.vector.reciprocal(dinv_pp[:], dsq[:])
            if phase <= 1:
                nc.sync.dma_start(out_d[0:128, 0:NBUK], dinv_pp[:])

            if not gcnb_zero:
                gb_ps = phaps.tile([128, TC], F32, space="PSUM", tag="gbps")
                for h in range(2):
                    nc.tensor.matmul(gb_ps[:, h * 512:(h + 1) * 512],
                                     lhsT=ones_f1[:],
                                     rhs=gbrow[:, h * 512:(h + 1) * 512].bitcast(F32),
                                     start=True, stop=True)
                nc.vector.tensor_copy(gcnbb[:], gb_ps[:])

        # ---- phase B: dense trunk per 512-node chunk
        from concourse.masks import make_identity
        if phase <= 1:
            nc.compile()
            return nc
        ident = const.tile([128, 128], F32)
        make_identity(nc, ident[:])

        xs_bounce = dram.tile([NP, TC], GF8)
        xs_full = dram.tile([NCORES * NP, TC], GF8)

        with tc.tile_pool(name="xst", bufs=2) as xst, \
             tc.tile_pool(name="phb", bufs=2) as phb, \
             tc.tile_pool(name="hsb", bufs=1) as hsb, \
             tc.tile_pool(name="hps", bufs=2, space="PSUM") as hps, \
             tc.tile_pool(name="stps", bufs=1, space="PSUM") as stps, \
             tc.tile_pool(name="bcps", bufs=1, space="PSUM") as bcps, \
             tc.tile_pool(name="xtps", bufs=1, space="PSUM") as xtps, \
             tc.tile_pool(name="trps", bufs=1, space="PSUM") as trps:
            for cc in range(NCH):
                nsl = slice(cc * 512, (cc + 1) * 512)
                stacks = [xst.tile([128, 512], F32R, tag=f"st{t}", name=f"st{t}") for t in range(T)]
                h_ps = {}
                h_sb = {}
                mu_ps = stps.tile([T, 512], F32, space="PSUM", tag="mu")
                ms_ps = stps.tile([T, 512], F32, space="PSUM", tag="ms")

                def post(t):
                    # h done in h_ps[t]: bias via ACT copy, then stats matmuls
                    hp = h_ps.pop(t)
                    h_sb[t] = hsb.tile([128, 512], F32R, tag=f"h{t}", name=f"hsb{t}")
                    nc.scalar.activation(h_sb[t][:], hp[:],
                                         mybir.ActivationFunctionType.Identity,
                                         bias=tcb_t[:, :1], scale=1.0)
                    hsq = phb.tile([128, 512], F32R, tag="hsq")
                    nc.scalar.activation(hsq[:], h_sb[t][:],
                                         mybir.ActivationFunctionType.Square)
                    nc.tensor.matmul(mu_ps[:], lhsT=onet[:, t, :], rhs=h_sb[t][:],
                                     start=(t == 0), stop=(t == T - 1))
                    nc.tensor.matmul(ms_ps[:], lhsT=onet[:, t, :], rhs=hsq[:],
                                     start=(t == 0), stop=(t == T - 1))

                for t in range(T):
                    st = stacks[t]
                    if t > 0:
                        nc.sync.dma_start(st[0:64, :], x_in[t - 1, :, nsl])
                    nc.sync.dma_start(st[64:128, :], x_in[t, :, nsl])
                    h_ps[t] = hps.tile([128, 512], F32, space="PSUM", tag="h", name=f"hps{t}")
                    if t == 0:
                        nc.tensor.matmul(h_ps[t][:], lhsT=ws01[64:128, :],
                                         rhs=st[64:128, :], start=True, stop=False)
                    else:
                        nc.tensor.matmul(h_ps[t][:], lhsT=ws01[:], rhs=st[:],
                                         start=True, stop=(t == T - 1))
                    if t > 0:
                        nc.tensor.matmul(h_ps[t - 1][:], lhsT=w2hi[64:128, :],
                                         rhs=st[64:128, :], start=False, stop=True)
                        post(t - 1)
                post(T - 1)

                # stats -> mu_n, rstd rows [T, 512]
                st_mu = phb.tile([T, 512], F32R, tag="stmu")
                nc.vector.tensor_scalar(out=st_mu[:], in0=mu_ps[:], scalar1=1.0 / CO,
                                        scalar2=None, op0=mybir.AluOpType.mult)
                msn = phb.tile([T, 512], F32, tag="msn")
                nc.vector.tensor_scalar(out=msn[:], in0=ms_ps[:], scalar1=1.0 / CO,
                                        scalar2=None, op0=mybir.AluOpType.mult)
                mu2 = phb.tile([T, 512], F32, tag="mu2")
                nc.vector.tensor_tensor(out=mu2[:], in0=st_mu[:], in1=st_mu[:],
                                        op=mybir.AluOpType.mult)
                var = phb.tile([T, 512], F32, tag="var")
                nc.vector.tensor_tensor(out=var[:], in0=msn[:], in1=mu2[:],
                                        op=mybir.AluOpType.subtract)
                sdv = phb.tile([T, 512], F32, tag="sdv")
                nc.scalar.activation(sdv[:], var[:], mybir.ActivationFunctionType.Sqrt,
                                     bias=eps_t[:, :1], scale=1.0)
                st_rs = phb.tile([T, 512], F32R, tag="strs")
                nc.vector.reciprocal(st_rs[:], sdv[:])

                for t in range(T):
                    # broadcast mu/rstd rows to all 128 partitions (K=8 matmul)
                    bc_ps = bcps.tile([128, TC], F32, space="PSUM", tag="bc")
                    nc.tensor.matmul(bc_ps[:, 0:512],
                                     lhsT=selr[:, t * 128:(t + 1) * 128],
                                     rhs=st_mu[:], start=True, stop=True)
                    nc.tensor.matmul(bc_ps[:, 512:1024],
                                     lhsT=selr[:, t * 128:(t + 1) * 128],
                                     rhs=st_rs[:], start=True, stop=True)
                    # fast ACT copy to SBUF releases the PSUM slot so the
                    # next t's broadcasts don't wait on this t's DVE chain
                    bcsb = phb.tile([128, TC], F32, tag="bcsb")
                    nc.scalar.activation(bcsb[:], bc_ps[:],
                                         mybir.ActivationFunctionType.Identity)
                    t1 = phb.tile([128, 512], F32, tag="t1")
                    nc.vector.tensor_tensor(out=t1[:], in0=h_sb[t][:],
                                            in1=bcsb[:, 0:512],
                                            op=mybir.AluOpType.subtract)
                    t2 = phb.tile([128, 512], F32, tag="t2")
                    nc.vector.tensor_tensor(out=t2[:], in0=t1[:], in1=bcsb[:, 512:1024],
                                            op=mybir.AluOpType.mult)
                    xn = phb.tile([128, 512], F32R, tag="xn")
                    nc.scalar.activation(xn[:], t2[:], mybir.ActivationFunctionType.Relu,
                                         bias=lnb_t[:, :1], scale=lng_t[:, :1])
                    xt_ps = xtps.tile([128, 512], F32, space="PSUM", tag="xt")
                    n_mm = 2 if rb2_zero else 3
                    nc.tensor.matmul(xt_ps[:], lhsT=gwt[:], rhs=xn[:],
                                     start=True, stop=False)
                    nc.tensor.matmul(xt_ps[:], lhsT=rwhi[64:128, :],
                                     rhs=stacks[t][64:128, :],
                                     start=False, stop=(n_mm == 2))
                    if not rb2_zero:
                        nc.tensor.matmul(xt_ps[:], lhsT=rb2t[:],
                                         rhs=ones512[:].bitcast(F32R),
                                         start=False, stop=True)
                    # transpose 128-col slices to node-major, scale by dinv
                    xt_sb = phb.tile([128, 512], F32, tag="xtsb")
                    nc.scalar.activation(xt_sb[:], xt_ps[:],
                                         mybir.ActivationFunctionType.Identity)
                    for s in range(4):
                        b = cc * 4 + s
                        tp = trps.tile([128, 128], F32, space="PSUM", tag="tp")
                        nc.tensor.transpose(tp[:], xt_sb[:, s * 128:(s + 1) * 128],
                                            ident[:])
                        nc.vector.tensor_scalar(
                            out=s1T[b][:, t * 128:(t + 1) * 128], in0=tp[:],
                            scalar1=dinv_pp[:, b:b + 1], scalar2=None,
                            op0=mybir.AluOpType.mult)
                # export this chunk's xs rows quantized to fp8
                for s in range(4):
                    b = cc * 4 + s
                    xq = phb.tile([128, TC], GF8, tag="xq")
                    nc.vector.tensor_copy(xq[:], s1T[b][:])
                    nc.sync.dma_start(
                        out=xs_bounce[b * 128:(b + 1) * 128, :], in_=xq[:])


        if phase <= 2:
            for b in range(NBUK):
                nc.sync.dma_start(out_d[b * 128:(b + 1) * 128, :], s1T[b][:])
            nc.compile()
            return nc

        # ---- AllGather: one per node chunk (overlaps with dense compute)
        if "coll" in skips:
            nc.sync.dma_start(xs_full[0:NP, :], xs_bounce[:])
        else:
            for i in range(NCH):
                nc.gpsimd.collective_compute(
                    "AllGather", mybir.AluOpType.bypass,
                    replica_groups=[list(range(NCORES))],
                    ins=[xs_bounce[i * 512:(i + 1) * 512, :].opt()],
                    outs=[xs_full[i * NCORES * 512:(i + 1) * NCORES * 512, :].opt()],
                )

        if phase <= 3:
            with tc.tile_pool(name="p3", bufs=2) as p3:
                for b in range(NBUK):
                    tmp3 = p3.tile([128, TC], BF16, tag="t3")
                    nc.sync.dma_start(tmp3[:], xs_full[b * 128:(b + 1) * 128, :])
                    t3f = p3.tile([128, TC], F32, tag="t3f")
                    nc.vector.tensor_copy(t3f[:], tmp3[:])
                    nc.sync.dma_start(out_d[b * 128:(b + 1) * 128, :], t3f[:])
            nc.compile()
            return nc

        # ---- phase C: gather + segment matmul + epilogue
        with tc.tile_pool(name="gat", bufs=2) as gat, \
             tc.tile_pool(name="sbld", bufs=12) as sbld, \
             tc.tile_pool(name="phc", bufs=2) as phc, \
             tc.tile_pool(name="outp", bufs=2) as outp, \
             tc.tile_pool(name="aggps", bufs=2, space="PSUM") as aggps:
            for b in range(NBUK):
                g_t = gat.tile([128, cpb, 1024], GF8, tag="g")
                if "gather" in skips:
                    nc.vector.memset(g_t[:], 0.0)
                else:
                    gsz = int(os.environ.get("KGATHER", "576"))
                    gch = gsz // 128
                    for gg in range(cpb // gch + (1 if cpb % gch else 0)):
                        j0 = gg * gch
                        j1 = min(cpb, j0 + gch)
                        nn = (j1 - j0) * 128
                        nc.gpsimd.dma_gather(
                            g_t[:, j0:j1, :], xs_full[:],
                            gidx_t[:, b * (cap // 16) + j0 * 8:
                                   b * (cap // 16) + j1 * 8],
                            nn, nn, 1024)
                agg = aggps.tile([128, TC], F32, space="PSUM", tag="agg")
                for j in range(cpb):
                    ch = b * cpb + j
                    s_t = sbld.tile([128, 128], GF8, tag="s2")
                    nc.vector.tensor_scalar(
                        out=s_t[:], in0=iota_t[:],
                        scalar1=colr_t[:, ch:ch + 1], scalar2=ew_soft[:, ch:ch + 1],
                        op0=mybir.AluOpType.is_equal, op1=mybir.AluOpType.mult)
                    for h in range(2):
                        nc.tensor.matmul(agg[:, h * 512:(h + 1) * 512],
                                         lhsT=s_t[:],
                                         rhs=g_t[:, j, h * 512:(h + 1) * 512],
                                         start=(j == 0), stop=(j == cpb - 1))
                # epilogue: out = relu(dinv*(agg/8192 + s1T) [+ gcn_b])
                t0 = phc.tile([128, TC], F32, tag="e0")
                nc.vector.tensor_scalar(out=t0[:], in0=agg[:],
                                        scalar1=1.0 / 8192.0, scalar2=None,
                                        op0=mybir.AluOpType.mult)
                t1 = phc.tile([128, TC], F32, tag="e1")
                nc.vector.tensor_tensor(out=t1[:], in0=t0[:], in1=s1T[b][:],
                                        op=mybir.AluOpType.add)
                o_t = outp.tile([128, TC], F32, tag="o")
                if gcnb_zero:
                    nc.scalar.activation(o_t[:], t1[:],
                                         mybir.ActivationFunctionType.Relu,
                                         scale=dinv_pp[:, b:b + 1], bias=0.0)
                else:
                    t2 = phc.tile([128, TC], F32, tag="e2")
                    nc.vector.tensor_scalar(out=t2[:], in0=t1[:],
                                            scalar1=dinv_pp[:, b:b + 1], scalar2=None,
                                            op0=mybir.AluOpType.mult)
                    t3 = phc.tile([128, TC], F32, tag="e3")
                    nc.vector.tensor_tensor(out=t3[:], in0=t2[:], in1=gcnbb[:],
                                            op=mybir.AluOpType.add)
                    nc.scalar.activation(o_t[:], t3[:],
                                         mybir.ActivationFunctionType.Relu)
                nc.sync.dma_start(out_d[b * 128:(b + 1) * 128, :], o_t[:])

    nc.compile()
    return nc


# ---------------------------------------------------------------- entry point

def kernel(**inputs):
    x = np.asarray(inputs["x"], np.float32)
    col = np.asarray(inputs["edge_index"][1], np.int64)
    # pick bucket capacity from the data
    core_of = col // PC
    rel = col - core_of * PC
    gid = core_of * NBUK + rel // 128
    maxb = np.bincount(gid, minlength=NCORES * NBUK).max()
    cpb = max(2, int(-(-int(maxb) // 128)))

    in_maps, flags = _prep(
        x, inputs["edge_index"], inputs["edge_weight"],
        np.asarray(inputs["tconv_w"], np.float32), np.asarray(inputs["tconv_b"], np.float32),
        np.asarray(inputs["ln_g"], np.float32), np.asarray(inputs["ln_b"], np.float32),
        np.asarray(inputs["res_w"], np.float32), np.asarray(inputs["res_b"], np.float32),
        np.asarray(inputs["gcn_w"], np.float32), np.asarray(inputs["gcn_b"], np.float32),
        cpb)

    key = (cpb, flags["gcnb_zero"], flags["rb2_zero"])
    if key not in _PROG_CACHE:
        _PROG_CACHE[key] = _build(cpb, flags["gcnb_zero"], flags["rb2_zero"])
    nc = _PROG_CACHE[key]

    trace = os.environ.get("KERNEL_TRACE", "0") == "1"
    res = run_bass_kernel_spmd(nc, in_maps, core_ids=list(range(NCORES)),
                               trace=trace)
    if trace:
        print(f"HW exec time: {res.exec_time_ns} ns")

    shards = [res.results[c]["out"][:PC] for c in range(NCORES)]
    full = np.concatenate(shards, axis=0)              # [N, T*CO]
    return np.ascontiguousarray(
        full.reshape(N, T, CO).transpose(1, 0, 2)).astype(np.float32)


if __name__ == "__main__":
    # standalone smoke: runs reference (CPU, subprocess via test.py) comparison
    import subprocess, sys
    sys.exit(subprocess.call([sys.executable, "test.py"]))


# revision 38
# speedup vs baseline: 1.0094x; 1.0094x over previous
"""Distributed Trainium2 (Bass/Tile) kernel for an enhanced ST-GCN layer.

Math (matches the jax reference):
  h   = LN(conv1d_t(x, k=3, pad=1) + tconv_b) * ln_g + ln_b ; relu
  res = x @ res_w.T + res_b
  xt  = (h + res) @ gcn_w.T          (res folded: xt += x @ (gcn_w@res_w).T + gcn_w@res_b)
  ew  = softmax(edge_weight); deg[v] = 1 + sum_{e: col=v} ew_e ; dinv = rsqrt(deg)
  out[v] = relu(dinv[v] * (sum_{e: col=v} ew_e * dinv[row_e] * xt[row_e]  + dinv[v]*xt[v]) + gcn_b)

Sharding: nodes split contiguously across 8 cores (2500/core, padded to 2560).
Each core computes its nodes' dense trunk in a channels-on-partitions layout
(fp32r matmuls), scales by dinv, transposes to node-major, AllGathers an fp8
copy of xs = dinv*xt (edge weights pre-scaled 2^13 to survive e4m3), then
gathers source rows for its (dest-sorted, bucketed) edges with gpsimd
dma_gather and reduces them with one-hot-style S matmuls; the fp32 self-term
and a 2^-13 descale are applied in the epilogue.
"""

import os
import numpy as np
import ml_dtypes

import concourse.bass as bass
import concourse.tile as tile
from contextlib import ExitStack
from concourse import bacc, mybir
from concourse.bass_utils import run_bass_kernel_spmd

F32 = mybir.dt.float32
F32R = mybir.dt.float32r
BF16 = mybir.dt.bfloat16
GF8 = mybir.dt.float8e4
I16 = mybir.dt.int16

NCORES = 8
N, E, T, CIN, CO = 20000, 160000, 8, 64, 128
PC = N // NCORES            # 2500 nodes per core
NP = 2560                   # padded per-core nodes (20 buckets x 128)
NBUK = NP // 128            # 20
NCH = NP // 512             # 5 node chunks of 512
TC = T * CO                 # 1024 features per node row

_PROG_CACHE = {}


# ---------------------------------------------------------------- host prep

def _prep(x, edge_index, edge_weight, tconv_w, tconv_b, ln_g, ln_b,
          res_w, res_b, gcn_w, gcn_b, cpb):
    cap = cpb * 128           # edge slots per bucket
    cc_per_core = NBUK * cpb  # chunks per core
    tot_ch = NCORES * cc_per_core

    row = np.asarray(edge_index[0], np.int64)
    col = np.asarray(edge_index[1], np.int64)
    ew = np.asarray(edge_weight, np.float32)

    core_of = col // PC
    rel = col - core_of * PC
    buk = rel // 128

    # slot arrays, global chunk order: core-major, bucket-major
    g_idx = np.zeros(tot_ch * 128, np.int16)           # gather row ids (pad 0)
    g_colr = np.full(tot_ch * 128, -1.0, np.float32)   # col rel to bucket (pad -1)
    g_ew = np.full(tot_ch * 128, -1e9, np.float32)     # raw edge weight (pad -1e9)

    order = np.lexsort((buk, core_of))
    srow, scol_rel, sew = row[order], (rel - buk * 128)[order], ew[order]
    scor, sbuk = core_of[order], buk[order]
    # per (core,bucket) group boundaries
    gid = scor * NBUK + sbuk
    counts = np.bincount(gid, minlength=NCORES * NBUK)
    assert counts.max() <= cap, f"bucket overflow: {counts.max()} > {cap}"
    starts = np.concatenate([[0], np.cumsum(counts)])[:-1]
    within = np.arange(len(order)) - starts[gid]
    slot = gid * cap + within
    # chunk-major AG layout: table row = chunk*(8*512) + core*512 + local%512
    sloc = srow % PC
    g_idx[slot] = ((sloc // 512) * (NCORES * 512) + (srow // PC) * 512
                   + sloc % 512).astype(np.int16)
    g_colr[slot] = scol_rel
    g_ew[slot] = sew

    # per-core device arrays
    in_maps = []
    w0t = np.ascontiguousarray(tconv_w[:, :, 0].T)  # [64,128]
    w1t = np.ascontiguousarray(tconv_w[:, :, 1].T)
    w2t = np.ascontiguousarray(tconv_w[:, :, 2].T)
    wstack01 = np.concatenate([w0t, w1t], axis=0)   # [128,128]
    w2thi = np.zeros((128, CO), np.float32); w2thi[64:] = w2t
    gcnwT = np.ascontiguousarray(gcn_w.T)           # [128,128]
    rw2 = gcn_w @ res_w                              # [128,64]
    rw2thi = np.zeros((128, CO), np.float32); rw2thi[64:] = rw2.T
    rb2 = (gcn_w @ res_b).astype(np.float32)        # [128]
    oneh = np.zeros((128, T, T), np.float32)
    for t in range(T):
        oneh[:, t, t] = 1.0
    # selrow[k, t*128+p] = 1 iff k == t  (K=8 broadcast matmul lhsT)
    selrow = np.zeros((T, T * 128), np.float32)
    for t in range(T):
        selrow[t, t * 128:(t + 1) * 128] = 1.0
    iota = np.tile(np.arange(128, dtype=np.float32), (128, 1))
    gcnb_row = np.tile(gcn_b.astype(np.float32), T)[None, :]   # [1,1024]

    ew_chunks = g_ew.reshape(tot_ch, 128).T          # [128, tot_ch]
    colr_chunks = g_colr.reshape(tot_ch, 128).T      # [128, tot_ch]
    # wrapped int16 idx per bucket: [16, cap/16] tiled to 128 partitions
    idx_b = g_idx.reshape(NCORES * NBUK, cap)
    idx_wrapped = np.stack([np.tile(b.reshape(-1, 16).T, (8, 1)) for b in idx_b])
    # [NCORES*NBUK, 128, cap/16] -> per core concat along free
    idx_wrapped = idx_wrapped.reshape(NCORES, NBUK, 128, cap // 16)
    idx_wrapped = np.ascontiguousarray(np.transpose(idx_wrapped, (0, 2, 1, 3))
                                       ).reshape(NCORES, 128, NBUK * cap // 16)

    x = np.asarray(x, np.float32)
    for c in range(NCORES):
        xs = np.zeros((T, CIN, NP), np.float32)
        xs[:, :, :PC] = x[:, c * PC:(c + 1) * PC, :].transpose(0, 2, 1)
        my = slice(c * cc_per_core, (c + 1) * cc_per_core)
        ew_mine = ew_chunks[:, my]
        ew_rest = np.concatenate([ew_chunks[:, :c * cc_per_core],
                                  ew_chunks[:, (c + 1) * cc_per_core:]], axis=1)
        in_maps.append(dict(
            x_in=xs,
            wstack01=wstack01, w2thi=w2thi, gcnwT=gcnwT, rw2thi=rw2thi,
            rb2col=rb2[None, :], oneh=oneh.reshape(128, T * T), selrow=selrow,
            iota=iota, gcnb_row=gcnb_row,
            lng=ln_g.astype(np.float32)[:, None], lnb=ln_b.astype(np.float32)[:, None],
            tcb=tconv_b.astype(np.float32)[:, None],
            ewl=np.ascontiguousarray(np.concatenate([ew_mine, ew_rest], axis=1)),
            colr=np.ascontiguousarray(colr_chunks[:, my]),
            gidx=np.ascontiguousarray(idx_wrapped[c]),
        ))
    flags = dict(
        gcnb_zero=bool(np.all(gcn_b == 0)),
        rb2_zero=bool(np.all(res_b == 0)),
    )
    return in_maps, flags


# ---------------------------------------------------------------- device code

def _build(cpb, gcnb_zero, rb2_zero):
    phase = int(os.environ.get("KPHASE", "4"))
    skips = set(os.environ.get("KSKIP", "").split(","))
    cap = cpb * 128
    cc_per_core = NBUK * cpb
    tot_ch = NCORES * cc_per_core

    nc = bacc.Bacc("TRN2", target_bir_lowering=False, debug=False,
                   num_devices=NCORES, num_swdge_queues=2)

    x_in = nc.dram_tensor("x_in", [T, CIN, NP], F32R, kind="ExternalInput")
    wstack01 = nc.dram_tensor("wstack01", [128, CO], F32R, kind="ExternalInput")
    w2thi = nc.dram_tensor("w2thi", [128, CO], F32R, kind="ExternalInput")
    gcnwT = nc.dram_tensor("gcnwT", [CO, CO], F32R, kind="ExternalInput")
    rw2thi = nc.dram_tensor("rw2thi", [128, CO], F32R, kind="ExternalInput")
    rb2col = nc.dram_tensor("rb2col", [1, CO], F32R, kind="ExternalInput")
    oneh = nc.dram_tensor("oneh", [128, T * T], F32R, kind="ExternalInput")
    selrow_d = nc.dram_tensor("selrow", [T, T * 128], F32R, kind="ExternalInput")
    iota_d = nc.dram_tensor("iota", [128, 128], F32, kind="ExternalInput")
    gcnb_row = nc.dram_tensor("gcnb_row", [1, TC], F32R, kind="ExternalInput")
    lng_d = nc.dram_tensor("lng", [CO, 1], F32, kind="ExternalInput")
    lnb_d = nc.dram_tensor("lnb", [CO, 1], F32, kind="ExternalInput")
    tcb_d = nc.dram_tensor("tcb", [CO, 1], F32, kind="ExternalInput")
    ewl_d = nc.dram_tensor("ewl", [128, tot_ch], F32, kind="ExternalInput")
    colr_d = nc.dram_tensor("colr", [128, cc_per_core], F32, kind="ExternalInput")
    gidx_d = nc.dram_tensor("gidx", [128, NBUK * cap // 16], I16, kind="ExternalInput")
    out_d = nc.dram_tensor("out", [NP, TC], F32, kind="ExternalOutput")

    with tile.TileContext(nc, num_cores=NCORES) as tc, ExitStack() as ctx:
        ctx.enter_context(nc.allow_low_precision(
            reason="fp32r rounding of matmul operands is intentional"))
        const = ctx.enter_context(tc.tile_pool(name="const", bufs=1))
        dram = ctx.enter_context(tc.tile_pool(name="dram", bufs=1, space="DRAM"))
        resi = ctx.enter_context(tc.tile_pool(name="resi", bufs=1))

        # ---- constants
        ws01 = const.tile([128, CO], F32R); nc.sync.dma_start(ws01[:], wstack01[:])
        w2hi = const.tile([128, CO], F32R); nc.sync.dma_start(w2hi[:], w2thi[:])
        gwt = const.tile([CO, CO], F32R); nc.sync.dma_start(gwt[:], gcnwT[:])
        rwhi = const.tile([128, CO], F32R); nc.sync.dma_start(rwhi[:], rw2thi[:])
        rb2t = const.tile([1, CO], F32R); nc.sync.dma_start(rb2t[:], rb2col[:])
        onet = const.tile([128, T, T], F32R); nc.sync.dma_start(onet[:], oneh[:].rearrange("p (a b) -> p a b", a=T))
        selr = const.tile([T, T * 128], F32R); nc.sync.dma_start(selr[:], selrow_d[:])
        iota_t = const.tile([128, 128], F32); nc.sync.dma_start(iota_t[:], iota_d[:])
        gbrow = const.tile([1, TC], F32R); nc.sync.dma_start(gbrow[:], gcnb_row[:])
        lng_t = const.tile([CO, 1], F32); nc.sync.dma_start(lng_t[:], lng_d[:])
        lnb_t = const.tile([CO, 1], F32); nc.sync.dma_start(lnb_t[:], lnb_d[:])
        tcb_t = const.tile([CO, 1], F32); nc.sync.dma_start(tcb_t[:], tcb_d[:])
        colr_t = const.tile([128, cc_per_core], F32); nc.sync.dma_start(colr_t[:], colr_d[:])
        gidx_t = const.tile([128, NBUK * cap // 16], I16); nc.sync.dma_start(gidx_t[:], gidx_d[:])
        ones_bf = const.tile([128, 1], BF16); nc.vector.memset(ones_bf[:], 1.0)
        ones_f1 = const.tile([1, 128], F32); nc.vector.memset(ones_f1[:], 1.0)
        ones512 = const.tile([1, 512], F32); nc.vector.memset(ones512[:], 1.0)
        eps_t = const.tile([T, 1], F32); nc.vector.memset(eps_t[:], 1e-5)
        one_t = const.tile([1, 1], F32); nc.vector.memset(one_t[:], 1.0)
        ew_soft = const.tile([128, cc_per_core], F32)
        dinv_pp = const.tile([128, NBUK], F32)
        # gcn_b broadcast [128, TC] (built below via K=1 matmuls) unless zero
        gcnbb = None if gcnb_zero else const.tile([128, TC], F32)

        # resident across phases
        s1T = [resi.tile([128, TC], F32, tag=f"s1T{b}", name=f"s1T{b}") for b in range(NBUK)]

        # ---- phase A: softmax + degree + dinv
        with tc.tile_pool(name="pha", bufs=2) as pha, \
             tc.tile_pool(name="pha1", bufs=1) as pha1, \
             tc.tile_pool(name="phaps", bufs=2, space="PSUM") as phaps:
            ewl_t = pha1.tile([128, tot_ch], F32)
            nc.sync.dma_start(ewl_t[:], ewl_d[:])
            ewx = pha1.tile([128, tot_ch], F32)
            nc.scalar.activation(ewx[:], ewl_t[:], mybir.ActivationFunctionType.Exp)
            rsum = pha1.tile([128, 1], F32)
            nc.vector.tensor_reduce(rsum[:], ewx[:], axis=mybir.AxisListType.X,
                                    op=mybir.AluOpType.add)
            ones_f = pha1.tile([128, 1], F32); nc.vector.memset(ones_f[:], 1.0)
            z_ps = phaps.tile([1, 1], F32, space="PSUM")
            nc.tensor.matmul(z_ps[:], lhsT=rsum[:], rhs=ones_f[:], start=True, stop=True)
            zinv = pha1.tile([1, 1], F32)
            nc.vector.reciprocal(zinv[:], z_ps[:])
            zb_ps = phaps.tile([128, 1], F32, space="PSUM")
            nc.tensor.matmul(zb_ps[:], lhsT=ones_f1[:], rhs=zinv[:],
                             start=True, stop=True)
            zinv_pp = pha1.tile([128, 1], F32)
            nc.vector.tensor_copy(zinv_pp[:], zb_ps[:])
            nc.vector.tensor_scalar(out=ew_soft[:], in0=ewx[:, :cc_per_core],
                                    scalar1=zinv_pp[:, :1], scalar2=8192.0,
                                    op0=mybir.AluOpType.mult,
                                    op1=mybir.AluOpType.mult)

            ones_t1 = pha1.tile([1, 1], F32)
            nc.vector.memset(ones_t1[:], 1.0)
            degcol = pha1.tile([128, NBUK], F32)
            for b in range(NBUK):
                dps = phaps.tile([1, 128], F32, space="PSUM", tag="dps")
                for j in range(cpb):
                    ch = b * cpb + j
                    s_t = pha.tile([128, 128], BF16, tag="sdeg")
                    nc.vector.tensor_scalar(
                        out=s_t[:], in0=iota_t[:],
                        scalar1=colr_t[:, ch:ch + 1], scalar2=ew_soft[:, ch:ch + 1],
                        op0=mybir.AluOpType.is_equal, op1=mybir.AluOpType.mult)
                    nc.tensor.matmul(dps[:], lhsT=ones_bf[:], rhs=s_t[:],
                                     start=(j == 0), stop=(j == cpb - 1))
                degsb = pha.tile([1, 128], F32, tag="degsb")
                nc.vector.tensor_copy(degsb[:], dps[:])
                dtp = phaps.tile([128, 1], F32, space="PSUM", tag="dtp")
                nc.tensor.transpose(dtp[:], degsb[:], ones_t1[:])
                nc.vector.tensor_copy(degcol[:, b:b + 1], dtp[:])
            # dinv = 1/sqrt(deg + 1)
            dsq = pha1.tile([128, NBUK], F32)
            nc.scalar.activation(dsq[:], degcol[:], mybir.ActivationFunctionType.Sqrt,
                                 bias=ones_f[:, :1], scale=1.0 / 8192.0)
            nc.vector.reciprocal(dinv_pp[:], dsq[:])
            if phase <= 1:
                nc.sync.dma_start(out_d[0:128, 0:NBUK], dinv_pp[:])

            if not gcnb_zero:
                gb_ps = phaps.tile([128, TC], F32, space="PSUM", tag="gbps")
                for h in range(2):
                    nc.tensor.matmul(gb_ps[:, h * 512:(h + 1) * 512],
                                     lhsT=ones_f1[:],
                                     rhs=gbrow[:, h * 512:(h + 1) * 512].bitcast(F32),
                                     start=True, stop=True)
                nc.vector.tensor_copy(gcnbb[:], gb_ps[:])

        # ---- phase B: dense trunk per 512-node chunk
        from concourse.masks import make_identity
        if phase <= 1:
            nc.compile()
            return nc
        ident = const.tile([128, 128], F32)
        make_identity(nc, ident[:])

        xs_bounce = dram.tile([NP, TC], GF8)
        xs_full = dram.tile([NCORES * NP, TC], GF8)

        with tc.tile_pool(name="xst", bufs=2) as xst, \
             tc.tile_pool(name="phb", bufs=2) as phb, \
             tc.tile_pool(name="hsb", bufs=1) as hsb, \
             tc.tile_pool(name="hps", bufs=2, space="PSUM") as hps, \
             tc.tile_pool(name="stps", bufs=1, space="PSUM") as stps, \
             tc.tile_pool(name="bcps", bufs=1, space="PSUM") as bcps, \
             tc.tile_pool(name="xtps", bufs=1, space="PSUM") as xtps, \
             tc.tile_pool(name="trps", bufs=1, space="PSUM") as trps:
            for cc in range(NCH):
                nsl = slice(cc * 512, (cc + 1) * 512)
                stacks = [xst.tile([128, 512], F32R, tag=f"st{t}", name=f"st{t}") for t in range(T)]
                h_ps = {}
                h_sb = {}
                mu_ps = stps.tile([T, 512], F32, space="PSUM", tag="mu")
                ms_ps = stps.tile([T, 512], F32, space="PSUM", tag="ms")

                def post(t):
                    # h done in h_ps[t]: bias via ACT copy, then stats matmuls
                    hp = h_ps.pop(t)
                    h_sb[t] = hsb.tile([128, 512], F32R, tag=f"h{t}", name=f"hsb{t}")
                    nc.scalar.activation(h_sb[t][:], hp[:],
                                         mybir.ActivationFunctionType.Identity,
                                         bias=tcb_t[:, :1], scale=1.0)
                    hsq = phb.tile([128, 512], F32R, tag="hsq")
                    nc.scalar.activation(hsq[:], h_sb[t][:],
                                         mybir.ActivationFunctionType.Square)
                    nc.tensor.matmul(mu_ps[:], lhsT=onet[:, t, :], rhs=h_sb[t][:],
                                     start=(t == 0), stop=(t == T - 1))
                    nc.tensor.matmul(ms_ps[:], lhsT=onet[:, t, :], rhs=hsq[:],
                                     start=(t == 0), stop=(t == T - 1))

                for t in range(T):
                    st = stacks[t]
                    if t > 0:
                        nc.sync.dma_start(st[0:64, :], x_in[t - 1, :, nsl])
                    nc.sync.dma_start(st[64:128, :], x_in[t, :, nsl])
                    h_ps[t] = hps.tile([128, 512], F32, space="PSUM", tag="h", name=f"hps{t}")
                    if t == 0:
                        nc.tensor.matmul(h_ps[t][:], lhsT=ws01[64:128, :],
                                         rhs=st[64:128, :], start=True, stop=False)
                    else:
                        nc.tensor.matmul(h_ps[t][:], lhsT=ws01[:], rhs=st[:],
                                         start=True, stop=(t == T - 1))
                    if t > 0:
                        nc.tensor.matmul(h_ps[t - 1][:], lhsT=w2hi[64:128, :],
                                         rhs=st[64:128, :], start=False, stop=True)
                        post(t - 1)
                post(T - 1)

                # stats -> mu_n, rstd rows [T, 512]
                st_mu = phb.tile([T, 512], F32R, tag="stmu")
                nc.vector.tensor_scalar(out=st_mu[:], in0=mu_ps[:], scalar1=1.0 / CO,
                                        scalar2=None, op0=mybir.AluOpType.mult)
                msn = phb.tile([T, 512], F32, tag="msn")
                nc.vector.tensor_scalar(out=msn[:], in0=ms_ps[:], scalar1=1.0 / CO,
                                        scalar2=None, op0=mybir.AluOpType.mult)
                mu2 = phb.tile([T, 512], F32, tag="mu2")
                nc.vector.tensor_tensor(out=mu2[:], in0=st_mu[:], in1=st_mu[:],
                                        op=mybir.AluOpType.mult)
                var = phb.tile([T, 512], F32, tag="var")
                nc.vector.tensor_tensor(out=var[:], in0=msn[:], in1=mu2[:],
                                        op=mybir.AluOpType.subtract)
                sdv = phb.tile([T, 512], F32, tag="sdv")
                nc.scalar.activation(sdv[:], var[:], mybir.ActivationFunctionType.Sqrt,
                                     bias=eps_t[:, :1], scale=1.0)
                st_rs = phb.tile([T, 512], F32R, tag="strs")
                nc.vector.reciprocal(st_rs[:], sdv[:])

                for t in range(T):
                    # broadcast mu/rstd rows to all 128 partitions (K=8 matmul)
                    bc_ps = bcps.tile([128, TC], F32, space="PSUM", tag="bc")
                    nc.tensor.matmul(bc_ps[:, 0:512],
                                     lhsT=selr[:, t * 128:(t + 1) * 128],
                                     rhs=st_mu[:], start=True, stop=True)
                    nc.tensor.matmul(bc_ps[:, 512:1024],
                                     lhsT=selr[:, t * 128:(t + 1) * 128],
                                     rhs=st_rs[:], start=True, stop=True)
                    t1 = phb.tile([128, 512], F32, tag="t1")
                    nc.vector.tensor_tensor(out=t1[:], in0=h_sb[t][:],
                                            in1=bc_ps[:, 0:512],
                                            op=mybir.AluOpType.subtract)
                    t2 = phb.tile([128, 512], F32, tag="t2")
                    nc.vector.tensor_tensor(out=t2[:], in0=t1[:], in1=bc_ps[:, 512:1024],
                                            op=mybir.AluOpType.mult)
                    xn = phb.tile([128, 512], F32R, tag="xn")
                    nc.scalar.activation(xn[:], t2[:], mybir.ActivationFunctionType.Relu,
                                         bias=lnb_t[:, :1], scale=lng_t[:, :1])
                    xt_ps = xtps.tile([128, 512], F32, space="PSUM", tag="xt")
                    n_mm = 2 if rb2_zero else 3
                    nc.tensor.matmul(xt_ps[:], lhsT=gwt[:], rhs=xn[:],
                                     start=True, stop=False)
                    nc.tensor.matmul(xt_ps[:], lhsT=rwhi[64:128, :],
                                     rhs=stacks[t][64:128, :],
                                     start=False, stop=(n_mm == 2))
                    if not rb2_zero:
                        nc.tensor.matmul(xt_ps[:], lhsT=rb2t[:],
                                         rhs=ones512[:].bitcast(F32R),
                                         start=False, stop=True)
                    # transpose 128-col slices to node-major, scale by dinv
                    xt_sb = phb.tile([128, 512], F32, tag="xtsb")
                    nc.scalar.activation(xt_sb[:], xt_ps[:],
                                         mybir.ActivationFunctionType.Identity)
                    for s in range(4):
                        b = cc * 4 + s
                        tp = trps.tile([128, 128], F32, space="PSUM", tag="tp")
                        nc.tensor.transpose(tp[:], xt_sb[:, s * 128:(s + 1) * 128],
                                            ident[:])
                        nc.vector.tensor_scalar(
                            out=s1T[b][:, t * 128:(t + 1) * 128], in0=tp[:],
                            scalar1=dinv_pp[:, b:b + 1], scalar2=None,
                            op0=mybir.AluOpType.mult)
                # export this chunk's xs rows quantized to fp8
                for s in range(4):
                    b = cc * 4 + s
                    xq = phb.tile([128, TC], GF8, tag="xq")
                    nc.vector.tensor_copy(xq[:], s1T[b][:])
                    nc.sync.dma_start(
                        out=xs_bounce[b * 128:(b + 1) * 128, :], in_=xq[:])


        if phase <= 2:
            for b in range(NBUK):
                nc.sync.dma_start(out_d[b * 128:(b + 1) * 128, :], s1T[b][:])
            nc.compile()
            return nc

        # ---- AllGather: one per node chunk (overlaps with dense compute)
        if "coll" in skips:
            nc.sync.dma_start(xs_full[0:NP, :], xs_bounce[:])
        else:
            for i in range(NCH):
                nc.gpsimd.collective_compute(
                    "AllGather", mybir.AluOpType.bypass,
                    replica_groups=[list(range(NCORES))],
                    ins=[xs_bounce[i * 512:(i + 1) * 512, :].opt()],
                    outs=[xs_full[i * NCORES * 512:(i + 1) * NCORES * 512, :].opt()],
                )

        if phase <= 3:
            with tc.tile_pool(name="p3", bufs=2) as p3:
                for b in range(NBUK):
                    tmp3 = p3.tile([128, TC], BF16, tag="t3")
                    nc.sync.dma_start(tmp3[:], xs_full[b * 128:(b + 1) * 128, :])
                    t3f = p3.tile([128, TC], F32, tag="t3f")
                    nc.vector.tensor_copy(t3f[:], tmp3[:])
                    nc.sync.dma_start(out_d[b * 128:(b + 1) * 128, :], t3f[:])
            nc.compile()
            return nc

        # ---- phase C: gather + segment matmul + epilogue
        with tc.tile_pool(name="gat", bufs=2) as gat, \
             tc.tile_pool(name="sbld", bufs=12) as sbld, \
             tc.tile_pool(name="phc", bufs=2) as phc, \
             tc.tile_pool(name="outp", bufs=2) as outp, \
             tc.tile_pool(name="aggps", bufs=2, space="PSUM") as aggps:
            for b in range(NBUK):
                g_t = gat.tile([128, cpb, 1024], GF8, tag="g")
                if "gather" in skips:
                    nc.vector.memset(g_t[:], 0.0)
                else:
                    gsz = int(os.environ.get("KGATHER", "576"))
                    gch = gsz // 128
                    for gg in range(cpb // gch + (1 if cpb % gch else 0)):
                        j0 = gg * gch
                        j1 = min(cpb, j0 + gch)
                        nn = (j1 - j0) * 128
                        nc.gpsimd.dma_gather(
                            g_t[:, j0:j1, :], xs_full[:],
                            gidx_t[:, b * (cap // 16) + j0 * 8:
                                   b * (cap // 16) + j1 * 8],
                            nn, nn, 1024)
                agg = aggps.tile([128, TC], F32, space="PSUM", tag="agg")
                for j in range(cpb):
                    ch = b * cpb + j
                    s_t = sbld.tile([128, 128], GF8, tag="s2")
                    nc.vector.tensor_scalar(
                        out=s_t[:], in0=iota_t[:],
                        scalar1=colr_t[:, ch:ch + 1], scalar2=ew_soft[:, ch:ch + 1],
                        op0=mybir.AluOpType.is_equal, op1=mybir.AluOpType.mult)
                    for h in range(2):
                        nc.tensor.matmul(agg[:, h * 512:(h + 1) * 512],
                                         lhsT=s_t[:],
                                         rhs=g_t[:, j, h * 512:(h + 1) * 512],
                                         start=(j == 0), stop=(j == cpb - 1))
                # epilogue: out = relu(dinv*(agg/8192 + s1T) [+ gcn_b])
                t0 = phc.tile([128, TC], F32, tag="e0")
                nc.vector.tensor_scalar(out=t0[:], in0=agg[:],
                                        scalar1=1.0 / 8192.0, scalar2=None,
                                        op0=mybir.AluOpType.mult)
                t1 = phc.tile([128, TC], F32, tag="e1")
                nc.vector.tensor_tensor(out=t1[:], in0=t0[:], in1=s1T[b][:],
                                        op=mybir.AluOpType.add)
                o_t = outp.tile([128, TC], F32, tag="o")
                if gcnb_zero:
                    nc.scalar.activation(o_t[:], t1[:],
                                         mybir.ActivationFunctionType.Relu,
                                         scale=dinv_pp[:, b:b + 1], bias=0.0)
                else:
                    t2 = phc.tile([128, TC], F32, tag="e2")
                    nc.vector.tensor_scalar(out=t2[:], in0=t1[:],
                                            scalar1=dinv_pp[:, b:b + 1], scalar2=None,
                                            op0=mybir.AluOpType.mult)
                    t3 = phc.tile([128, TC], F32, tag="e3")
                    nc.vector.tensor_tensor(out=t3[:], in0=t2[:], in1=gcnbb[:],
                                            op=mybir.AluOpType.add)
                    nc.scalar.activation(o_t[:], t3[:],
                                         mybir.ActivationFunctionType.Relu)
                nc.sync.dma_start(out_d[b * 128:(b + 1) * 128, :], o_t[:])

    nc.compile()
    return nc


# ---------------------------------------------------------------- entry point

def kernel(**inputs):
    x = np.asarray(inputs["x"], np.float32)
    col = np.asarray(inputs["edge_index"][1], np.int64)
    # pick bucket capacity from the data
    core_of = col // PC
    rel = col - core_of * PC
    gid = core_of * NBUK + rel // 128
    maxb = np.bincount(gid, minlength=NCORES * NBUK).max()
    cpb = max(2, int(-(-int(maxb) // 128)))

    in_maps, flags = _prep(
        x, inputs["edge_index"], inputs["edge_weight"],
        np.asarray(inputs["tconv_w"], np.float32), np.asarray(inputs["tconv_b"], np.float32),
        np.asarray(inputs["ln_g"], np.float32), np.asarray(inputs["ln_b"], np.float32),
        np.asarray(inputs["res_w"], np.float32), np.asarray(inputs["res_b"], np.float32),
        np.asarray(inputs["gcn_w"], np.float32), np.asarray(inputs["gcn_b"], np.float32),
        cpb)

    key = (cpb, flags["gcnb_zero"], flags["rb2_zero"])
    if key not in _PROG_CACHE:
        _PROG_CACHE[key] = _build(cpb, flags["gcnb_zero"], flags["rb2_zero"])
    nc = _PROG_CACHE[key]

    trace = os.environ.get("KERNEL_TRACE", "0") == "1"
    res = run_bass_kernel_spmd(nc, in_maps, core_ids=list(range(NCORES)),
                               trace=trace)
    if trace:
        print(f"HW exec time: {res.exec_time_ns} ns")

    shards = [res.results[c]["out"][:PC] for c in range(NCORES)]
    full = np.concatenate(shards, axis=0)              # [N, T*CO]
    return np.ascontiguousarray(
        full.reshape(N, T, CO).transpose(1, 0, 2)).astype(np.float32)


if __name__ == "__main__":
    # standalone smoke: runs reference (CPU, subprocess via test.py) comparison
    import subprocess, sys
    sys.exit(subprocess.call([sys.executable, "test.py"]))
